# revision 12
# baseline (speedup 1.0000x reference)
"""Trainium2 Bass kernel: batched causal attention (B=4, S=4096, E=256, f32).

Sharding: 2 cores per batch element (4 pairs). Within a pair, the key/value
rows are split even/odd at 128-row tile granularity; both cores process all
4096 query rows of their batch against their 2048 K/V rows.  This keeps the
SPMD instruction stream identical across cores (only data differs) and
perfectly load-balances the causal structure.  Partial (P@V, rowsum) results
are merged across each pair with per-pair ReduceScatters (bf16 payload),
after which each core normalizes and writes half the batch rows.

Compute layout (per core):
  X^T, Z^T via PE transposes (bf16) -> Q^T = WqT @ X^T (scaled 1/sqrt(E),
  +bq), K^T = WkT @ Z^T (bk dropped: softmax shift-invariant), V = Z^T
  (stationary) @ WvT (bv folded in at the end: attn rows sum to 1).
  Scores per tile are computed transposed: S^T[k,q] = K^T(stationary) . Q^T,
  so exp(PSUM)->SBUF directly yields P^T (bf16) for the PV matmul.
  Rowsums via a ones-stationary matmul accumulated in PSUM.
  Pairs are processed most-expensive-first so collectives overlap compute.
"""

import numpy as np

B = 4
S = 4096
E = 256
SK = S // 2          # K/V rows per core
KT = SK // 128       # 16 local k-tiles
NCHUNK = S // 512    # 8 q-chunks of 512
F = 512              # q free dim per chunk
NPOST = NCHUNK // 2  # post-phase chunks per core

_COMPILED = {}


def _build():
    import concourse.bass as bass
    import concourse.tile as tile
    from concourse import mybir, bacc
    from concourse.masks import make_identity

    f32 = mybir.dt.float32
    bf16 = mybir.dt.bfloat16
    Exp = mybir.ActivationFunctionType.Exp
    Copy = mybir.ActivationFunctionType.Copy
    Ident = mybir.ActivationFunctionType.Identity

    nc = bacc.Bacc("TRN2", target_bir_lowering=False, debug=False,
                   enable_asserts=True, num_devices=8)

    x_ext = nc.dram_tensor("x", [S, E], f32, kind="ExternalInput")
    z_ext = nc.dram_tensor("z", [SK, E], f32, kind="ExternalInput")
    wq_ext = nc.dram_tensor("wq", [E, E], f32, kind="ExternalInput")
    wk_ext = nc.dram_tensor("wk", [E, E], f32, kind="ExternalInput")
    wv_ext = nc.dram_tensor("wv", [E, E], f32, kind="ExternalInput")
    bqs_ext = nc.dram_tensor("bqs", [E], f32, kind="ExternalInput")  # bq/sqrt(E)
    bv_ext = nc.dram_tensor("bv", [E], f32, kind="ExternalInput")
    masks_ext = nc.dram_tensor("masks", [2, 128, F], f32, kind="ExternalInput")
    ones_ext = nc.dram_tensor("ones", [128, 128], f32, kind="ExternalInput")
    out_ext = nc.dram_tensor("out", [S // 2, E], f32, kind="ExternalOutput")

    with tile.TileContext(nc) as tc:
        with tc.tile_pool(name="singles", bufs=1) as singles, \
             tc.tile_pool(name="dram", bufs=1, space="DRAM") as dram:
            # ---- constants -------------------------------------------------
            ident_bf = singles.tile([128, 128], bf16)
            make_identity(nc, ident_bf[:])
            ones_r = singles.tile([128, 128], bf16)
            nc.gpsimd.dma_start(out=ones_r[:], in_=ones_ext[:])
            maskt = singles.tile([128, 2, F], bf16)
            nc.gpsimd.dma_start(out=maskt[:], in_=masks_ext.ap().rearrange("m p f -> p m f"))
            bqs = singles.tile([128, 2], f32)
            for ft in range(2):
                nc.sync.dma_start(out=bqs[:, ft:ft + 1],
                                  in_=bqs_ext[128 * ft:128 * (ft + 1)].rearrange("(p one) -> p one", one=1))
            bv_bc = singles.tile([128, E], f32)
            nc.sync.dma_start(
                out=bv_bc[:],
                in_=bass.AP(tensor=bv_ext, offset=0, ap=[[0, 128], [1, E]]))

            # ---- weights: W^T[e', f] in SBUF (bf16), via PE transposes -----
            wT = {}
            with tc.tile_pool(name="wload", bufs=2) as wload, \
                 tc.tile_pool(name="ps_w", bufs=2, space="PSUM") as ps_w:
                for wname, wext in (("q", wq_ext), ("k", wk_ext), ("v", wv_ext)):
                    for et in range(2):
                        wT[wname, et] = singles.tile([128, E], bf16, name=f"wT_{wname}{et}")
                    for ft in range(2):
                        wnat = wload.tile([128, E], bf16, name="wnat")
                        nc.gpsimd.dma_start(out=wnat[:],
                                            in_=wext[128 * ft:128 * (ft + 1), :])
                        pst = ps_w.tile([128, E], bf16, name="pstw")
                        for et in range(2):
                            nc.tensor.transpose(pst[:, 128 * et:128 * (et + 1)],
                                                wnat[:, 128 * et:128 * (et + 1)],
                                                ident_bf[:])
                        for et in range(2):
                            nc.vector.tensor_copy(
                                out=wT[wname, et][:, 128 * ft:128 * (ft + 1)],
                                in_=pst[:, 128 * et:128 * (et + 1)])

            # ---- big persistent SBUF tensors -------------------------------
            qT = [singles.tile([128, S], bf16, name=f"qT{i}", tag=f"qT{i}") for i in range(2)]
            kT = [singles.tile([128, SK], bf16, name=f"kT{i}", tag=f"kT{i}") for i in range(2)]
            v_sb = singles.tile([128, KT, E], bf16, tag="v_sb")

            partials_in = dram.tile([NPOST, 2, 257, F], bf16)
            partials_out = dram.tile([NPOST, 257, F], bf16)

            with tc.tile_pool(name="nat", bufs=3) as nat, \
                 tc.tile_pool(name="trsb", bufs=4) as trsb, \
                 tc.tile_pool(name="pT", bufs=4) as pTp, \
                 tc.tile_pool(name="partsb", bufs=2) as partsb, \
                 tc.tile_pool(name="post", bufs=2) as post, \
                 tc.tile_pool(name="ps_tr", bufs=1, space="PSUM") as ps_tr, \
                 tc.tile_pool(name="ps_mm", bufs=1, space="PSUM") as ps_mm, \
                 tc.tile_pool(name="ps_s", bufs=3, space="PSUM") as ps_s, \
                 tc.tile_pool(name="ps_o", bufs=1, space="PSUM") as ps_o:

                def load_transposed(src_ap, dst_tiles):
                    x_nat = nat.tile([128, 4, E], bf16, tag="nat", name="x_nat")
                    nc.gpsimd.dma_start(out=x_nat[:],
                                        in_=src_ap.rearrange("(t p) e -> p t e", p=128))
                    for et in range(2):
                        pst = ps_tr.tile([128, F], bf16, tag="ps_tr", name="pst")
                        for t in range(4):
                            nc.tensor.transpose(
                                pst[:, 128 * t:128 * (t + 1)],
                                x_nat[:, t, 128 * et:128 * (et + 1)], ident_bf[:])
                        nc.vector.tensor_copy(out=dst_tiles[et][:], in_=pst[:])

                for sc in range(4):
                    zT = [trsb.tile([128, F], bf16, tag="xT", name=f"zT{et}")
                          for et in range(2)]
                    load_transposed(z_ext[512 * sc:512 * (sc + 1), :], zT)
                    for ft in range(2):
                        psk = ps_mm.tile([128, F], f32, tag="ps_mm", name="psk")
                        for et in range(2):
                            nc.tensor.matmul(psk[:], wT["k", et][:, 128 * ft:128 * (ft + 1)],
                                             zT[et][:], start=(et == 0), stop=(et == 1))
                        nc.vector.tensor_copy(out=kT[ft][:, 512 * sc:512 * (sc + 1)],
                                              in_=psk[:])
                    for t in range(4):
                        psv = ps_mm.tile([128, E], f32, tag="ps_mm", name="psv",
                                         padded_shape=[128, 512])
                        for et in range(2):
                            nc.tensor.matmul(psv[:], zT[et][:, 128 * t:128 * (t + 1)],
                                             wT["v", et][:], start=(et == 0), stop=(et == 1))
                        nc.vector.tensor_copy(out=v_sb[:, 4 * sc + t, :], in_=psv[:])

                def proj_q(j):
                    xT = [trsb.tile([128, F], bf16, tag="xT", name=f"xT{et}")
                          for et in range(2)]
                    load_transposed(x_ext[512 * j:512 * (j + 1), :], xT)
                    for ft in range(2):
                        psq = ps_mm.tile([128, F], f32, tag="ps_mm", name="psq")
                        for et in range(2):
                            nc.tensor.matmul(psq[:], wT["q", et][:, 128 * ft:128 * (ft + 1)],
                                             xT[et][:], start=(et == 0), stop=(et == 1))
                        nc.scalar.activation(out=qT[ft][:, 512 * j:512 * (j + 1)],
                                             in_=psq[:], func=Ident,
                                             bias=bqs[:, ft:ft + 1],
                                             scale=1.0 / float(np.sqrt(E)))

                def attn_chunk(j, pair, half):
                    nkt = 2 * (j + 1)
                    pso = ps_o.tile([128, 3 * F], f32, tag="ps_o", name="pso")
                    psr = pso[:, 2 * F:3 * F]
                    for ll in range(nkt):
                        pss = ps_s.tile([128, F], f32, tag="ps_s", name="pss")
                        for et in range(2):
                            nc.tensor.matmul(pss[:], kT[et][:, 128 * ll:128 * (ll + 1)],
                                             qT[et][:, 512 * j:512 * (j + 1)],
                                             start=(et == 0), stop=(et == 1))
                        pT = pTp.tile([128, F], bf16, tag="pT", name="pT")
                        nc.scalar.activation(out=pT[:], in_=pss[:], func=Exp)
                        if ll >= nkt - 2:
                            nc.vector.tensor_mul(pT[:], pT[:],
                                                 maskt[:, ll - (nkt - 2), :])
                        for ft in range(2):
                            nc.tensor.matmul(pso[:, F * ft:F * (ft + 1)],
                                             v_sb[:, ll, 128 * ft:128 * (ft + 1)],
                                             pT[:], start=(ll == 0), stop=(ll == nkt - 1),
                                             skip_group_check=True)
                        nc.tensor.matmul(psr, ones_r[:], pT[:],
                                         start=(ll == 0), stop=(ll == nkt - 1),
                                         skip_group_check=True)
                    po_sb = partsb.tile([128, 2 * F], bf16, tag="po_sb", name="po_sb")
                    nc.scalar.activation(out=po_sb[:], in_=pso[:, 0:2 * F], func=Copy)
                    pr_sb = partsb.tile([1, F], bf16, tag="pr_sb", name="pr_sb")
                    nc.vector.tensor_copy(out=pr_sb[:], in_=psr[0:1, :])
                    for ft in range(2):
                        nc.sync.dma_start(
                            out=partials_in[pair, half, 128 * ft:128 * (ft + 1), :],
                            in_=po_sb[:, F * ft:F * (ft + 1)])
                    nc.sync.dma_start(out=partials_in[pair, half, 256, :], in_=pr_sb[0:1, :])

                def post_chunk(c):
                    oT_sb = post.tile([128, 2 * F], bf16, tag="oT_sb", name="oT_sb")
                    for ft in range(2):
                        nc.sync.dma_start(out=oT_sb[:, F * ft:F * (ft + 1)],
                                          in_=partials_out[c, 128 * ft:128 * (ft + 1), :])
                    rs_ld = post.tile([128, 4], bf16, tag="rs_ld", name="rs_ld")
                    nc.sync.dma_start(out=rs_ld[:],
                                      in_=partials_out[c, 256, :].rearrange("(t p) -> p t", p=128))
                    rs_t = post.tile([128, 4], f32, tag="rs_t", name="rs_t")
                    nc.vector.reciprocal(out=rs_t[:], in_=rs_ld[:])
                    onat = post.tile([128, 4, E], f32, tag="onat", name="onat")
                    for t in range(4):
                        pst = ps_tr.tile([128, E], bf16, tag="ps_tr", name="pstp",
                                         padded_shape=[128, 512])
                        for ft in range(2):
                            nc.tensor.transpose(
                                pst[:, 128 * ft:128 * (ft + 1)],
                                oT_sb[:, F * ft + 128 * t:F * ft + 128 * (t + 1)],
                                ident_bf[:])
                        nc.scalar.activation(out=onat[:, t, :], in_=pst[:],
                                             func=Copy, scale=rs_t[:, t:t + 1])
                        nc.vector.tensor_add(onat[:, t, :], onat[:, t, :], bv_bc[:])
                    nc.sync.dma_start(
                        out=out_ext[512 * c:512 * (c + 1), :].rearrange(
                            "(t p) e -> p t e", p=128),
                        in_=onat[:])

                prev = None
                for pair in (3, 2, 1, 0):
                    proj_q(pair)
                    proj_q(NPOST + pair)
                    attn_chunk(pair, pair, 0)
                    attn_chunk(NPOST + pair, pair, 1)
                    nc.gpsimd.collective_compute(
                        "ReduceScatter", mybir.AluOpType.add,
                        replica_groups=[[0, 1], [2, 3], [4, 5], [6, 7]],
                        ins=[partials_in[pair].opt()],
                        outs=[partials_out[pair].opt()])
                    if prev is not None:
                        post_chunk(prev)
                    prev = pair
                post_chunk(prev)

    nc.compile()
    return nc


def _get_nc():
    if "nc" not in _COMPILED:
        _COMPILED["nc"] = _build()
    return _COMPILED["nc"]


def kernel(X, Z, mask, Wq, bq, Wk, bk, Wv, bv):
    X = np.asarray(X, dtype=np.float32)
    Z = np.asarray(Z, dtype=np.float32)
    mask_np = np.asarray(mask)

    causal = bool(np.array_equal(
        mask_np != 0, np.tril(np.ones((S, S), dtype=bool))))
    if not causal:
        return _numpy_ref(X, Z, mask_np, Wq, bq, Wk, bk, Wv, bv)

    from concourse.bass_utils import run_bass_kernel_spmd

    nc = _get_nc()

    Wq = np.ascontiguousarray(Wq, dtype=np.float32)
    Wk = np.ascontiguousarray(Wk, dtype=np.float32)
    Wv = np.ascontiguousarray(Wv, dtype=np.float32)
    bqs = (np.asarray(bq, dtype=np.float32) / np.float32(np.sqrt(E))).copy()
    bv = np.ascontiguousarray(bv, dtype=np.float32)
    ones = np.ones((128, 128), dtype=np.float32)

    # masks per parity: last-2 local k-tiles of each chunk; keep iff y >= x+d
    y = np.arange(F)[None, :]
    x = np.arange(128)[:, None]
    masks_par = []
    for p in range(2):
        m = np.stack([(y >= x + 128 * p).astype(np.float32),
                      (y >= x + 256 + 128 * p).astype(np.float32)])
        masks_par.append(np.ascontiguousarray(m))

    in_maps = []
    for c in range(8):
        b, p = c // 2, c % 2
        zb = Z[b].reshape(S // 128, 128, E)
        z_shard = np.ascontiguousarray(zb[p::2].reshape(SK, E))
        in_maps.append({
            "x": np.ascontiguousarray(X[b]),
            "z": z_shard,
            "wq": Wq, "wk": Wk, "wv": Wv,
            "bqs": bqs, "bv": bv,
            "masks": masks_par[p],
            "ones": ones,
        })

    res = run_bass_kernel_spmd(nc, in_maps, core_ids=list(range(8)))

    out = np.empty((B, S, E), dtype=np.float32)
    for b in range(B):
        out[b, :S // 2] = res.results[2 * b]["out"]
        out[b, S // 2:] = res.results[2 * b + 1]["out"]
    return out


def _numpy_ref(X, Z, mask, Wq, bq, Wk, bk, Wv, bv):
    q = np.einsum("bse,fe->bsf", X, Wq) + bq
    k = np.einsum("bse,fe->bsf", Z, Wk) + bk
    v = np.einsum("bse,fe->bsf", Z, Wv) + bv
    s = np.einsum("bqe,bke->bqk", q, k) / np.sqrt(np.float32(X.shape[-1]))
    s = np.where(mask == 0, -np.inf, s)
    s = s - s.max(axis=-1, keepdims=True)
    p = np.exp(s)
    p /= p.sum(axis=-1, keepdims=True)
    return np.einsum("bqk,bke->bqe", p, v).astype(np.float32)


# revision 13
# speedup vs baseline: 1.1348x; 1.1348x over previous
"""Trainium2 Bass kernel: batched causal attention (B=4, S=4096, E=256, f32).

Sharding: 2 cores per batch element (4 pairs). Within a pair, the key/value
rows are split even/odd at 128-row tile granularity; both cores process all
4096 query rows of their batch against their 2048 K/V rows.  This keeps the
SPMD instruction stream identical across cores (only data differs) and
perfectly load-balances the causal structure.  Partial (P@V, rowsum) results
are merged across each pair with per-pair ReduceScatters (bf16 payload),
after which each core normalizes and writes half the batch rows.

Compute layout (per core):
  X^T, Z^T via PE transposes (bf16) -> Q^T = WqT @ X^T (scaled 1/sqrt(E),
  +bq), K^T = WkT @ Z^T (bk dropped: softmax shift-invariant), V = Z^T
  (stationary) @ WvT (bv folded in at the end: attn rows sum to 1).
  Scores per tile are computed transposed: S^T[k,q] = K^T(stationary) . Q^T,
  so exp(PSUM)->SBUF directly yields P^T (bf16) for the PV matmul.
  Rowsums via a ones-stationary matmul accumulated in PSUM.
  Pairs are processed most-expensive-first so collectives overlap compute.
"""

import numpy as np

B = 4
S = 4096
E = 256
SK = S // 2          # K/V rows per core
KT = SK // 128       # 16 local k-tiles
NCHUNK = S // 512    # 8 q-chunks of 512
F = 512              # q free dim per chunk
NPOST = NCHUNK // 2  # post-phase chunks per core

_COMPILED = {}


def _build():
    import concourse.bass as bass
    import concourse.tile as tile
    from concourse import mybir, bacc
    from concourse.masks import make_identity

    f32 = mybir.dt.float32
    bf16 = mybir.dt.bfloat16
    Exp = mybir.ActivationFunctionType.Exp
    Copy = mybir.ActivationFunctionType.Copy
    Ident = mybir.ActivationFunctionType.Identity

    nc = bacc.Bacc("TRN2", target_bir_lowering=False, debug=False,
                   enable_asserts=True, num_devices=8)

    x_ext = nc.dram_tensor("x", [S, E], f32, kind="ExternalInput")
    z_ext = nc.dram_tensor("z", [SK, E], f32, kind="ExternalInput")
    wq_ext = nc.dram_tensor("wq", [E, E], f32, kind="ExternalInput")
    wk_ext = nc.dram_tensor("wk", [E, E], f32, kind="ExternalInput")
    wv_ext = nc.dram_tensor("wv", [E, E], f32, kind="ExternalInput")
    bqs_ext = nc.dram_tensor("bqs", [E], f32, kind="ExternalInput")  # bq/sqrt(E)
    bv_ext = nc.dram_tensor("bv", [E], f32, kind="ExternalInput")
    masks_ext = nc.dram_tensor("masks", [2, 128, F], f32, kind="ExternalInput")
    ones_ext = nc.dram_tensor("ones", [128, 128], f32, kind="ExternalInput")
    out_ext = nc.dram_tensor("out", [S // 2, E], f32, kind="ExternalOutput")

    with tile.TileContext(nc) as tc:
        with tc.tile_pool(name="singles", bufs=1) as singles, \
             tc.tile_pool(name="dram", bufs=1, space="DRAM") as dram:
            # ---- constants -------------------------------------------------
            ident_bf = singles.tile([128, 128], bf16)
            make_identity(nc, ident_bf[:])
            ones_r = singles.tile([128, 128], bf16)
            nc.gpsimd.dma_start(out=ones_r[:], in_=ones_ext[:])
            maskt = singles.tile([128, 2, F], bf16)
            nc.gpsimd.dma_start(out=maskt[:], in_=masks_ext.ap().rearrange("m p f -> p m f"))
            bqs = singles.tile([128, 2], f32)
            for ft in range(2):
                nc.sync.dma_start(out=bqs[:, ft:ft + 1],
                                  in_=bqs_ext[128 * ft:128 * (ft + 1)].rearrange("(p one) -> p one", one=1))
            bv_bc = singles.tile([128, E], f32)
            nc.sync.dma_start(
                out=bv_bc[:],
                in_=bass.AP(tensor=bv_ext, offset=0, ap=[[0, 128], [1, E]]))

            # ---- weights: W^T[e', f] in SBUF (bf16), via PE transposes -----
            wT = {}
            with tc.tile_pool(name="wload", bufs=2) as wload, \
                 tc.tile_pool(name="ps_w", bufs=2, space="PSUM") as ps_w:
                for wname, wext in (("q", wq_ext), ("k", wk_ext), ("v", wv_ext)):
                    for et in range(2):
                        wT[wname, et] = singles.tile([128, E], bf16, name=f"wT_{wname}{et}")
                    for ft in range(2):
                        wnat = wload.tile([128, E], bf16, name="wnat")
                        nc.gpsimd.dma_start(out=wnat[:],
                                            in_=wext[128 * ft:128 * (ft + 1), :])
                        pst = ps_w.tile([128, E], bf16, name="pstw")
                        for et in range(2):
                            nc.tensor.transpose(pst[:, 128 * et:128 * (et + 1)],
                                                wnat[:, 128 * et:128 * (et + 1)],
                                                ident_bf[:])
                        for et in range(2):
                            nc.vector.tensor_copy(
                                out=wT[wname, et][:, 128 * ft:128 * (ft + 1)],
                                in_=pst[:, 128 * et:128 * (et + 1)])

            # ---- big persistent SBUF tensors -------------------------------
            qT = [singles.tile([128, S], bf16, name=f"qT{i}", tag=f"qT{i}") for i in range(2)]
            kT = [singles.tile([128, SK], bf16, name=f"kT{i}", tag=f"kT{i}") for i in range(2)]
            v_sb = singles.tile([128, KT, E], bf16, tag="v_sb")

            partials_in = dram.tile([NPOST, 2, 257, F], bf16)
            partials_out = dram.tile([NPOST, 257, F], bf16)

            with tc.tile_pool(name="nat", bufs=3) as nat, \
                 tc.tile_pool(name="trsb", bufs=4) as trsb, \
                 tc.tile_pool(name="ps_tr", bufs=3, space="PSUM") as ps_tr, \
                 tc.tile_pool(name="ps_mm", bufs=3, space="PSUM") as ps_mm:

                def load_transposed(src_ap, dst_tiles):
                    x_nat = nat.tile([128, 4, E], bf16, tag="nat", name="x_nat")
                    nc.gpsimd.dma_start(out=x_nat[:],
                                        in_=src_ap.rearrange("(t p) e -> p t e", p=128))
                    for et in range(2):
                        pst = ps_tr.tile([128, F], bf16, tag="ps_tr", name="pst")
                        for t in range(4):
                            nc.tensor.transpose(
                                pst[:, 128 * t:128 * (t + 1)],
                                x_nat[:, t, 128 * et:128 * (et + 1)], ident_bf[:])
                        nc.vector.tensor_copy(out=dst_tiles[et][:], in_=pst[:])

                for sc in range(4):
                    zT = [trsb.tile([128, F], bf16, tag="xT", name=f"zT{et}")
                          for et in range(2)]
                    load_transposed(z_ext[512 * sc:512 * (sc + 1), :], zT)
                    for ft in range(2):
                        psk = ps_mm.tile([128, F], f32, tag="ps_mm", name="psk")
                        for et in range(2):
                            nc.tensor.matmul(psk[:], wT["k", et][:, 128 * ft:128 * (ft + 1)],
                                             zT[et][:], start=(et == 0), stop=(et == 1))
                        nc.vector.tensor_copy(out=kT[ft][:, 512 * sc:512 * (sc + 1)],
                                              in_=psk[:])
                    for t in range(4):
                        psv = ps_mm.tile([128, E], f32, tag="ps_mm", name="psv",
                                         padded_shape=[128, 512])
                        for et in range(2):
                            nc.tensor.matmul(psv[:], zT[et][:, 128 * t:128 * (t + 1)],
                                             wT["v", et][:], start=(et == 0), stop=(et == 1))
                        nc.vector.tensor_copy(out=v_sb[:, 4 * sc + t, :], in_=psv[:])

                for j in (3, 7, 2, 6, 1, 5, 0, 4):
                    xT = [trsb.tile([128, F], bf16, tag="xT", name=f"xT{et}")
                          for et in range(2)]
                    load_transposed(x_ext[512 * j:512 * (j + 1), :], xT)
                    for ft in range(2):
                        psq = ps_mm.tile([128, F], f32, tag="ps_mm", name="psq")
                        for et in range(2):
                            nc.tensor.matmul(psq[:], wT["q", et][:, 128 * ft:128 * (ft + 1)],
                                             xT[et][:], start=(et == 0), stop=(et == 1))
                        nc.scalar.activation(out=qT[ft][:, 512 * j:512 * (j + 1)],
                                             in_=psq[:], func=Ident,
                                             bias=bqs[:, ft:ft + 1],
                                             scale=1.0 / float(np.sqrt(E)))

            with tc.tile_pool(name="pT", bufs=8) as pTp, \
                 tc.tile_pool(name="partsb", bufs=4) as partsb, \
                 tc.tile_pool(name="post", bufs=2) as post, \
                 tc.tile_pool(name="ps_s", bufs=4, space="PSUM") as ps_s, \
                 tc.tile_pool(name="ps_o", bufs=1, space="PSUM") as ps_o, \
                 tc.tile_pool(name="ps_rs", bufs=1, space="PSUM") as ps_rs, \
                 tc.tile_pool(name="ps_po", bufs=1, space="PSUM") as ps_po:

                def attn_chunk(j, pair, half):
                    nkt = 2 * (j + 1)
                    pso = ps_o.tile([128, 2 * F], f32, tag="ps_o", name="pso")
                    psr = ps_rs.tile([128, F], f32, tag="ps_rs", name="psr")
                    for ll2 in range(0, nkt, 2):
                        pTs = []
                        for d in range(2):
                            ll = ll2 + d
                            pss = ps_s.tile([128, F], f32, tag="ps_s", name="pss")
                            for et in range(2):
                                nc.tensor.matmul(pss[:], kT[et][:, 128 * ll:128 * (ll + 1)],
                                                 qT[et][:, 512 * j:512 * (j + 1)],
                                                 start=(et == 0), stop=(et == 1))
                            pT = pTp.tile([128, F], bf16, tag="pT", name="pT")
                            nc.scalar.activation(out=pT[:], in_=pss[:], func=Exp)
                            if ll >= nkt - 2:
                                nc.vector.tensor_mul(pT[:], pT[:],
                                                     maskt[:, ll - (nkt - 2), :])
                            for ft in range(2):
                                nc.tensor.matmul(pso[:, F * ft:F * (ft + 1)],
                                                 v_sb[:, ll, 128 * ft:128 * (ft + 1)],
                                                 pT[:], start=(ll == 0), stop=(ll == nkt - 1),
                                                 skip_group_check=True)
                            pTs.append(pT)
                        pT2 = pTp.tile([128, F], bf16, tag="pT", name="pT2")
                        nc.vector.tensor_add(pT2[:], pTs[0][:], pTs[1][:])
                        nc.tensor.matmul(psr[:], ones_r[:], pT2[:],
                                         start=(ll2 == 0), stop=(ll2 == nkt - 2),
                                         skip_group_check=True)
                    po_sb = partsb.tile([128, 2 * F], bf16, tag="po_sb", name="po_sb")
                    nc.scalar.activation(out=po_sb[:], in_=pso[:], func=Copy)
                    pr_sb = partsb.tile([1, F], bf16, tag="pr_sb", name="pr_sb")
                    nc.vector.tensor_copy(out=pr_sb[:], in_=psr[0:1, :])
                    for ft in range(2):
                        nc.sync.dma_start(
                            out=partials_in[pair, half, 128 * ft:128 * (ft + 1), :],
                            in_=po_sb[:, F * ft:F * (ft + 1)])
                    nc.sync.dma_start(out=partials_in[pair, half, 256, :], in_=pr_sb[0:1, :])

                def post_chunk(c):
                    oT_sb = post.tile([128, 2 * F], bf16, tag="oT_sb", name="oT_sb")
                    for ft in range(2):
                        nc.sync.dma_start(out=oT_sb[:, F * ft:F * (ft + 1)],
                                          in_=partials_out[c, 128 * ft:128 * (ft + 1), :])
                    rs_ld = post.tile([128, 4], bf16, tag="rs_ld", name="rs_ld")
                    nc.sync.dma_start(out=rs_ld[:],
                                      in_=partials_out[c, 256, :].rearrange("(t p) -> p t", p=128))
                    rs_t = post.tile([128, 4], f32, tag="rs_t", name="rs_t")
                    nc.vector.reciprocal(out=rs_t[:], in_=rs_ld[:])
                    onat = post.tile([128, 4, E], f32, tag="onat", name="onat")
                    for t in range(4):
                        pst = ps_po.tile([128, E], bf16, tag="ps_po", name="pstp")
                        for ft in range(2):
                            nc.tensor.transpose(
                                pst[:, 128 * ft:128 * (ft + 1)],
                                oT_sb[:, F * ft + 128 * t:F * ft + 128 * (t + 1)],
                                ident_bf[:])
                        nc.scalar.activation(out=onat[:, t, :], in_=pst[:],
                                             func=Copy, scale=rs_t[:, t:t + 1])
                        nc.vector.tensor_add(onat[:, t, :], onat[:, t, :], bv_bc[:])
                    nc.sync.dma_start(
                        out=out_ext[512 * c:512 * (c + 1), :].rearrange(
                            "(t p) e -> p t e", p=128),
                        in_=onat[:])

                prev = None
                for pair in (3, 2, 1, 0):
                    attn_chunk(pair, pair, 0)
                    attn_chunk(NPOST + pair, pair, 1)
                    nc.gpsimd.collective_compute(
                        "ReduceScatter", mybir.AluOpType.add,
                        replica_groups=[[0, 1], [2, 3], [4, 5], [6, 7]],
                        ins=[partials_in[pair].opt()],
                        outs=[partials_out[pair].opt()])
                    if prev is not None:
                        post_chunk(prev)
                    prev = pair
                post_chunk(prev)

    nc.compile()
    return nc


def _get_nc():
    if "nc" not in _COMPILED:
        _COMPILED["nc"] = _build()
    return _COMPILED["nc"]


def kernel(X, Z, mask, Wq, bq, Wk, bk, Wv, bv):
    X = np.asarray(X, dtype=np.float32)
    Z = np.asarray(Z, dtype=np.float32)
    mask_np = np.asarray(mask)

    causal = bool(np.array_equal(
        mask_np != 0, np.tril(np.ones((S, S), dtype=bool))))
    if not causal:
        return _numpy_ref(X, Z, mask_np, Wq, bq, Wk, bk, Wv, bv)

    from concourse.bass_utils import run_bass_kernel_spmd

    nc = _get_nc()

    Wq = np.ascontiguousarray(Wq, dtype=np.float32)
    Wk = np.ascontiguousarray(Wk, dtype=np.float32)
    Wv = np.ascontiguousarray(Wv, dtype=np.float32)
    bqs = (np.asarray(bq, dtype=np.float32) / np.float32(np.sqrt(E))).copy()
    bv = np.ascontiguousarray(bv, dtype=np.float32)
    ones = np.ones((128, 128), dtype=np.float32)

    # masks per parity: last-2 local k-tiles of each chunk; keep iff y >= x+d
    y = np.arange(F)[None, :]
    x = np.arange(128)[:, None]
    masks_par = []
    for p in range(2):
        m = np.stack([(y >= x + 128 * p).astype(np.float32),
                      (y >= x + 256 + 128 * p).astype(np.float32)])
        masks_par.append(np.ascontiguousarray(m))

    in_maps = []
    for c in range(8):
        b, p = c // 2, c % 2
        zb = Z[b].reshape(S // 128, 128, E)
        z_shard = np.ascontiguousarray(zb[p::2].reshape(SK, E))
        in_maps.append({
            "x": np.ascontiguousarray(X[b]),
            "z": z_shard,
            "wq": Wq, "wk": Wk, "wv": Wv,
            "bqs": bqs, "bv": bv,
            "masks": masks_par[p],
            "ones": ones,
        })

    res = run_bass_kernel_spmd(nc, in_maps, core_ids=list(range(8)))

    out = np.empty((B, S, E), dtype=np.float32)
    for b in range(B):
        out[b, :S // 2] = res.results[2 * b]["out"]
        out[b, S // 2:] = res.results[2 * b + 1]["out"]
    return out


def _numpy_ref(X, Z, mask, Wq, bq, Wk, bk, Wv, bv):
    q = np.einsum("bse,fe->bsf", X, Wq) + bq
    k = np.einsum("bse,fe->bsf", Z, Wk) + bk
    v = np.einsum("bse,fe->bsf", Z, Wv) + bv
    s = np.einsum("bqe,bke->bqk", q, k) / np.sqrt(np.float32(X.shape[-1]))
    s = np.where(mask == 0, -np.inf, s)
    s = s - s.max(axis=-1, keepdims=True)
    p = np.exp(s)
    p /= p.sum(axis=-1, keepdims=True)
    return np.einsum("bqk,bke->bqe", p, v).astype(np.float32)
